# revision 1
# baseline (speedup 1.0000x reference)
"""Trainium2 Bass kernel for nn_Net_91268055040039 (dense_mlp).

Computes out[b] = sum_{t,p} x[b,t,p] * |W[t,p]| * fc1_w[0, t*P+p] + fc1_b
  x: [32, 400, 10000] f32, W: [400, 10000] f32, fc1_w: [1, 4000000] f32.

Strategy: shard the reduction dim T=400 into 8 slices of 50 rows (64MB of x +
4MB of params per core, vs 64+32MB for batch sharding). Per core the 500000
reduction elements per batch are padded to 128*3907 and laid out
partition-major ON THE HOST, so each SBUF partition's data for consecutive
batches is contiguous in HBM. DMA then moves 8MB chunks with 62.5KB
contiguous per-partition runs (~397 GB/s measured on this setup, vs 181 GB/s
for 16KB runs - descriptor overhead dominates short runs).

Per core:
  v = |W_shard| * fc1_shard              (ACT abs + DVE mult, in-place)
  for b in 32: acc[:, b] = reduce_add(x_tile_b * v)   (one fused DVE
        scalar_tensor_tensor with accum_out per batch; tensor_tensor_reduce
        crashes this HW/runtime build)
  psum[1, 32] = ones[128,1].T @ acc[128,32]           (PE partition reduction)
Host sums the 8 per-core partials and adds fc1_b.
"""

import numpy as np

import concourse.bass as bass
import concourse.bacc as bacc
import concourse.mybir as mybir
from concourse.tile import TileContext
from concourse.bass_utils import run_bass_kernel_spmd

B, T, P = 32, 400, 10000
NCORES = 8
TS = T // NCORES          # 50 T-rows per core
K = TS * P                # 500000 reduction elements per core per batch
PART = 128
FREE = 3907               # ceil(K / PART); 128*3907 = 500096 (96 zero pad)
KPAD = PART * FREE
CHUNK = 4                 # batches per DMA: 4 * 3907 * 4B = 62.5KB per row
NCHUNKS = B // CHUNK
F32 = mybir.dt.float32

# Set by the test harness to capture an NTFF profile; harmless when False.
TRACE = False
LAST_RESULT = None


def build_program() -> bass.Bass:
    # Bacc (not raw Bass): its compile() splits multi-sem waits into separate
    # instructions - this neuronxcc build allows only 1 sync-wait per inst.
    nc = bacc.Bacc()
    xs = nc.declare_dram_parameter("xs", [PART, B * FREE], F32, isOutput=False)
    # wf[:, :FREE] = W shard, wf[:, FREE:] = fc1 shard (one DMA for both).
    wf = nc.declare_dram_parameter("wf", [PART, 2 * FREE], F32, isOutput=False)
    out = nc.declare_dram_parameter("out", [1, B], F32, isOutput=True)

    with TileContext(nc) as tc:
        with (
            tc.tile_pool(name="const", bufs=1) as cpool,
            tc.tile_pool(name="xp", bufs=2) as xpool,
            tc.tile_pool(name="psum", bufs=1, space="PSUM") as ppool,
        ):
            # Params on the sync/HWDGE ring so the gpsimd/SWDGE ring starts
            # streaming x immediately.
            wft = cpool.tile([PART, 2 * FREE], F32)
            nc.sync.dma_start(out=wft, in_=wf[:, :])
            # v = |W| * fc1, computed in place over the W half of wft.
            v = wft[:, :FREE]
            nc.scalar.activation(
                out=v, in_=v, func=mybir.ActivationFunctionType.Abs
            )
            nc.vector.tensor_tensor(
                out=v, in0=v, in1=wft[:, FREE:], op=mybir.AluOpType.mult
            )

            ones = cpool.tile([PART, 1], F32)
            nc.vector.memset(ones, 1.0)
            acc = cpool.tile([PART, B], F32)
            scratch = cpool.tile([PART, FREE], F32)

            for g in range(NCHUNKS):
                xt = xpool.tile([PART, CHUNK * FREE], F32, tag="xt")
                nc.gpsimd.dma_start(
                    out=xt, in_=xs[:, g * CHUNK * FREE : (g + 1) * CHUNK * FREE]
                )
                for c in range(CHUNK):
                    b = g * CHUNK + c
                    # Fused multiply + free-dim reduce in one DVE pass:
                    # scratch = (x_b bypass 0) mult v; acc[:, b] = sum(scratch)
                    nc.vector.scalar_tensor_tensor(
                        out=scratch,
                        in0=xt[:, c * FREE : (c + 1) * FREE],
                        scalar=0.0,
                        in1=v,
                        op0=mybir.AluOpType.bypass,
                        op1=mybir.AluOpType.mult,
                        accum_out=acc[:, b : b + 1],
                    )

            ps = ppool.tile([1, B], F32)
            nc.tensor.matmul(out=ps, lhsT=ones, rhs=acc, start=True, stop=True)
            res = cpool.tile([1, B], F32)
            nc.scalar.copy(res, ps)
            nc.sync.dma_start(out=out[:, :], in_=res)
    nc.finalize()
    return nc


def _to_partition_major(flat: np.ndarray) -> np.ndarray:
    """[N, K] row-major -> [PART, N*FREE] where each partition's rows for
    consecutive N are adjacent (N along the middle axis)."""
    n = flat.shape[0]
    padded = np.zeros((n, KPAD), dtype=np.float32)
    padded[:, :K] = flat
    # [n, PART, FREE] -> [PART, n, FREE] -> [PART, n*FREE]
    return np.ascontiguousarray(
        padded.reshape(n, PART, FREE).transpose(1, 0, 2)
    ).reshape(PART, n * FREE)


def make_in_maps(x: np.ndarray, W: np.ndarray, fc1_w: np.ndarray):
    x = np.asarray(x, dtype=np.float32)
    W = np.asarray(W, dtype=np.float32)
    fc1_w = np.asarray(fc1_w, dtype=np.float32)
    fc1_flat = fc1_w.reshape(T, P)
    in_maps = []
    for c in range(NCORES):
        t0 = c * TS
        xs = _to_partition_major(x[:, t0 : t0 + TS, :].reshape(B, K))
        ws = _to_partition_major(W[t0 : t0 + TS, :].reshape(1, K))
        fs = _to_partition_major(fc1_flat[t0 : t0 + TS, :].reshape(1, K))
        in_maps.append({"xs": xs, "wf": np.concatenate([ws, fs], axis=1)})
    return in_maps


def kernel(x, W, fc1_w, fc1_b):
    global LAST_RESULT
    nc = build_program()
    in_maps = make_in_maps(x, W, fc1_w)
    res = run_bass_kernel_spmd(
        nc, in_maps, core_ids=list(range(NCORES)), trace=TRACE
    )
    LAST_RESULT = res
    partial = np.zeros(B, dtype=np.float64)
    for r in res.results:
        partial += r["out"][0].astype(np.float64)
    out = partial.astype(np.float32) + np.float32(np.asarray(fc1_b).reshape(-1)[0])
    return out.reshape(B, 1).astype(np.float32)



# revision 2
# speedup vs baseline: 1.3621x; 1.3621x over previous
"""Trainium2 Bass kernel for nn_Net_91268055040039 (dense_mlp).

Computes out[b] = sum_{t,p} x[b,t,p] * |W[t,p]| * fc1_w[0, t*P+p] + fc1_b
  x: [32, 400, 10000] f32, W: [400, 10000] f32, fc1_w: [1, 4000000] f32.

Strategy (v2, fp16): shard the reduction dim T=400 into 8 slices of 50 rows.
x/W/fc1 are cast to fp16 on the host (halves HBM traffic, the binding
resource: ~358-390 GB/s per core; quantization rel-err ~4e-3, gate is 2e-2)
and laid out partition-major so DMA runs are contiguous per partition.
FREE is padded 3907->3908 so every per-batch slice is 4B-aligned (required
for DVE 2x perf mode).

Per core (32 MB x + 2 MB params):
  params DMA first on the SWDGE ring (fast), then x in descending chunks
  [8,8,8,5,2,1] batches double-buffered on the same ring.
  v = |W_shard| * fc1_shard   (ACT abs + DVE mult, fp16 in place)
  per batch b: dot(x_b, v) two ways (measurement A/B, both correct):
    b < 16:  DVE tensor_tensor mult (fp16 2x) -> scratch;
             ACT activation(Copy) with accum_out -> acc[:, b]
    b >= 16: DVE scalar_tensor_tensor fused mult+accum -> acc[:, b]
  psum[1, 32] = ones[128,1].T @ acc[128,32]   (PE partition reduction)
Host sums the 8 per-core partials in f64 and adds fc1_b.
"""

import numpy as np

import concourse.bass as bass
import concourse.bacc as bacc
import concourse.mybir as mybir
from concourse.tile import TileContext
from concourse.bass_utils import run_bass_kernel_spmd

B, T, P = 32, 400, 10000
NCORES = 8
TS = T // NCORES          # 50 T-rows per core
K = TS * P                # 500000 reduction elements per core per batch
PART = 128
FREE = 3908               # ceil(K/128)=3907, padded to even for 4B alignment
KPAD = PART * FREE        # 500224 (224 zero pad)
CHUNKS = [8, 8, 8, 5, 2, 1]   # batches per DMA; descending tail for low lag
CHUNK_MAX = max(CHUNKS)
N_TT_ACT = 16             # batches [0,16) take the TT+ACT path; rest fused stt
F16 = mybir.dt.float16
F32 = mybir.dt.float32

# Set by the test harness to capture an NTFF profile; harmless when False.
TRACE = False
LAST_RESULT = None


def build_program() -> bass.Bass:
    # Bacc (not raw Bass): its compile() splits multi-sem waits into separate
    # instructions - this neuronxcc build allows only 1 sync-wait per inst.
    nc = bacc.Bacc()
    xs = nc.declare_dram_parameter("xs", [PART, B * FREE], F16, isOutput=False)
    # wf[:, :FREE] = W shard, wf[:, FREE:] = fc1 shard (one DMA for both).
    wf = nc.declare_dram_parameter("wf", [PART, 2 * FREE], F16, isOutput=False)
    out = nc.declare_dram_parameter("out", [1, B], F32, isOutput=True)

    with TileContext(nc) as tc:
        with (
            tc.tile_pool(name="const", bufs=1) as cpool,
            tc.tile_pool(name="xp", bufs=2) as xpool,
            tc.tile_pool(name="sp", bufs=2) as spool,
            tc.tile_pool(name="psum", bufs=1, space="PSUM") as ppool,
        ):
            # Params first on the SWDGE ring: 2MB ~ 5us, so v is ready well
            # before the first x chunk lands (the HWDGE weights queue measured
            # only 259 GB/s with gaps in the previous version).
            wft = cpool.tile([PART, 2 * FREE], F16)
            nc.gpsimd.dma_start(out=wft, in_=wf[:, :])
            # v = |W| * fc1, computed in place over the W half of wft.
            v = wft[:, :FREE]
            nc.scalar.activation(
                out=v, in_=v, func=mybir.ActivationFunctionType.Abs
            )
            nc.vector.tensor_tensor(
                out=v, in0=v, in1=wft[:, FREE:], op=mybir.AluOpType.mult
            )

            ones = cpool.tile([PART, 1], F32)
            nc.vector.memset(ones, 1.0)
            acc = cpool.tile([PART, B], F32)
            dump = cpool.tile([PART, FREE], F16)

            b0 = 0
            for nb in CHUNKS:
                xt = xpool.tile([PART, CHUNK_MAX * FREE], F16, tag="xt")
                nc.gpsimd.dma_start(
                    out=xt[:, : nb * FREE],
                    in_=xs[:, b0 * FREE : (b0 + nb) * FREE],
                )
                for c in range(nb):
                    b = b0 + c
                    xin = xt[:, c * FREE : (c + 1) * FREE]
                    if b < N_TT_ACT:
                        # DVE 2x multiply; ACT does the free-dim reduce.
                        sc = spool.tile([PART, FREE], F16, tag="sc")
                        nc.vector.tensor_tensor(
                            out=sc, in0=xin, in1=v, op=mybir.AluOpType.mult
                        )
                        nc.scalar.activation(
                            out=dump,
                            in_=sc,
                            func=mybir.ActivationFunctionType.Copy,
                            accum_out=acc[:, b : b + 1],
                        )
                    else:
                        # Fused multiply + free-dim reduce in one DVE pass.
                        sc = spool.tile([PART, FREE], F16, tag="sc")
                        nc.vector.scalar_tensor_tensor(
                            out=sc,
                            in0=xin,
                            scalar=0.0,
                            in1=v,
                            op0=mybir.AluOpType.bypass,
                            op1=mybir.AluOpType.mult,
                            accum_out=acc[:, b : b + 1],
                        )
                b0 += nb

            ps = ppool.tile([1, B], F32)
            nc.tensor.matmul(out=ps, lhsT=ones, rhs=acc, start=True, stop=True)
            res = cpool.tile([1, B], F32)
            nc.scalar.copy(res, ps)
            nc.sync.dma_start(out=out[:, :], in_=res)
    nc.finalize()
    return nc


def _to_partition_major(flat: np.ndarray) -> np.ndarray:
    """[N, K] (f16) row-major -> [PART, N*FREE] where each partition's rows
    for consecutive N are adjacent in HBM."""
    n = flat.shape[0]
    padded = np.zeros((n, KPAD), dtype=np.float16)
    padded[:, :K] = flat
    # [n, PART, FREE] -> [PART, n, FREE] -> [PART, n*FREE]
    return np.ascontiguousarray(
        padded.reshape(n, PART, FREE).transpose(1, 0, 2)
    ).reshape(PART, n * FREE)


def make_in_maps(x: np.ndarray, W: np.ndarray, fc1_w: np.ndarray):
    x16 = np.asarray(x).astype(np.float16)
    W16 = np.asarray(W).astype(np.float16)
    f16 = np.asarray(fc1_w).astype(np.float16).reshape(T, P)
    in_maps = []
    for c in range(NCORES):
        t0 = c * TS
        xs = _to_partition_major(x16[:, t0 : t0 + TS, :].reshape(B, K))
        ws = _to_partition_major(W16[t0 : t0 + TS, :].reshape(1, K))
        fs = _to_partition_major(f16[t0 : t0 + TS, :].reshape(1, K))
        in_maps.append({"xs": xs, "wf": np.concatenate([ws, fs], axis=1)})
    return in_maps


def kernel(x, W, fc1_w, fc1_b):
    global LAST_RESULT
    nc = build_program()
    in_maps = make_in_maps(x, W, fc1_w)
    res = run_bass_kernel_spmd(
        nc, in_maps, core_ids=list(range(NCORES)), trace=TRACE
    )
    LAST_RESULT = res
    partial = np.zeros(B, dtype=np.float64)
    for r in res.results:
        partial += r["out"][0].astype(np.float64)
    out = partial.astype(np.float32) + np.float32(np.asarray(fc1_b).reshape(-1)[0])
    return out.reshape(B, 1).astype(np.float32)


# revision 3
# speedup vs baseline: 1.6213x; 1.1903x over previous
"""Trainium2 Bass kernel for nn_Net_91268055040039 (dense_mlp).

Computes out[b] = sum_{t,p} x[b,t,p] * |W[t,p]| * fc1_w[0, t*P+p] + fc1_b
  x: [32, 400, 10000] f32, W: [400, 10000] f32, fc1_w: [1, 4000000] f32.

Strategy (v3, fp16 + balanced DVE/ACT lanes): shard T=400 into 8 slices of 50
rows. x/W/fc1 cast to fp16 on host (halves HBM traffic, the binding resource;
quantization rel-err ~4e-3 vs the 2e-2 gate). FREE padded 3907->3908 so every
per-batch slice is 4B-aligned (DVE 2x perf-mode requirement).

Measured per-batch costs (FREE=3908 fp16): DVE tensor_tensor 2.2us (2x mode),
DVE scalar_tensor_tensor 4.3us (no 2x uop), ACT activation reduce 3.55+0.28us
(1x). DMA supplies ~2.6-2.8us/batch. A single fused-stt pipeline is DVE-bound
(137us); TT+ACT-only is ACT-bound (123us). Balance: 23 batches take
TT (DVE) + reduce (ACT), 9 batches take fused stt (DVE) -> both lanes ~89us,
just under the DMA stream time.

x streams in 17 uniform small chunks (15x2 + 2x1 batches), each its own dense
2MB dram block, 6-deep buffered: uniform chunks keep the compute pipeline fed
(big trailing chunks serialize their whole compute after the last byte), and
the final 1-batch chunks + fused-stt last batch minimize the tail.
Params ride the same SWDGE ring first (HWDGE weights queue measured slow).

  psum[1, 32] = ones[128,1].T @ acc[128,32]   (PE partition reduction)
Host sums the 8 per-core partials in f64 and adds fc1_b.
"""

import numpy as np

import concourse.bass as bass
import concourse.bacc as bacc
import concourse.mybir as mybir
from concourse.tile import TileContext
from concourse.bass_utils import run_bass_kernel_spmd

B, T, P = 32, 400, 10000
NCORES = 8
TS = T // NCORES          # 50 T-rows per core
K = TS * P                # 500000 reduction elements per core per batch
PART = 128
FREE = 3908               # ceil(K/128)=3907, padded to even for 4B alignment
KPAD = PART * FREE        # 500224 (224 zero pad)
CHUNKS = [2] * 15 + [1, 1]    # batches per DMA chunk (sum = 32)
# Batches on the fused-stt path (DVE-only, 4.3us); the rest take
# TT (DVE 2.2us) + ACT reduce (3.84us). 23/9 split balances the two lanes.
STT_BATCHES = {3, 7, 11, 15, 19, 23, 27, 30, 31}
F16 = mybir.dt.float16
F32 = mybir.dt.float32

# Set by the test harness to capture an NTFF profile; harmless when False.
TRACE = False
LAST_RESULT = None


def build_program() -> bass.Bass:
    # Bacc (not raw Bass): its compile() splits multi-sem waits into separate
    # instructions - this neuronxcc build allows only 1 sync-wait per inst.
    nc = bacc.Bacc()
    xcs = [
        nc.declare_dram_parameter(f"xs{g}", [PART, nb * FREE], F16, isOutput=False)
        for g, nb in enumerate(CHUNKS)
    ]
    # wf[:, :FREE] = W shard, wf[:, FREE:] = fc1 shard (one DMA for both).
    wf = nc.declare_dram_parameter("wf", [PART, 2 * FREE], F16, isOutput=False)
    out = nc.declare_dram_parameter("out", [1, B], F32, isOutput=True)

    with TileContext(nc) as tc:
        with (
            tc.tile_pool(name="const", bufs=1) as cpool,
            tc.tile_pool(name="xp", bufs=6) as xpool,
            tc.tile_pool(name="sp", bufs=2) as spool,
            tc.tile_pool(name="psum", bufs=1, space="PSUM") as ppool,
        ):
            # Params first on the SWDGE ring (~6us), so v is ready before the
            # first x chunk lands.
            wft = cpool.tile([PART, 2 * FREE], F16)
            nc.gpsimd.dma_start(out=wft, in_=wf[:, :])
            # v = |W| * fc1, computed in place over the W half of wft.
            v = wft[:, :FREE]
            nc.scalar.activation(
                out=v, in_=v, func=mybir.ActivationFunctionType.Abs
            )
            nc.vector.tensor_tensor(
                out=v, in0=v, in1=wft[:, FREE:], op=mybir.AluOpType.mult
            )

            ones = cpool.tile([PART, 1], F32)
            nc.vector.memset(ones, 1.0)
            acc = cpool.tile([PART, B], F32)
            dump = cpool.tile([PART, FREE], F16)

            b0 = 0
            for g, nb in enumerate(CHUNKS):
                xt = xpool.tile([PART, 2 * FREE], F16, tag="xt")
                nc.gpsimd.dma_start(out=xt[:, : nb * FREE], in_=xcs[g][:, :])
                for c in range(nb):
                    b = b0 + c
                    xin = xt[:, c * FREE : (c + 1) * FREE]
                    if b in STT_BATCHES:
                        # Fused multiply + free-dim reduce in one DVE pass.
                        sc = spool.tile([PART, FREE], F16, tag="sc")
                        nc.vector.scalar_tensor_tensor(
                            out=sc,
                            in0=xin,
                            scalar=0.0,
                            in1=v,
                            op0=mybir.AluOpType.bypass,
                            op1=mybir.AluOpType.mult,
                            accum_out=acc[:, b : b + 1],
                        )
                    else:
                        # DVE 2x multiply; ACT does the free-dim reduce.
                        sc = spool.tile([PART, FREE], F16, tag="sc")
                        nc.vector.tensor_tensor(
                            out=sc, in0=xin, in1=v, op=mybir.AluOpType.mult
                        )
                        nc.scalar.activation(
                            out=dump,
                            in_=sc,
                            func=mybir.ActivationFunctionType.Copy,
                            accum_out=acc[:, b : b + 1],
                        )
                b0 += nb

            ps = ppool.tile([1, B], F32)
            nc.tensor.matmul(out=ps, lhsT=ones, rhs=acc, start=True, stop=True)
            res = cpool.tile([1, B], F32)
            nc.scalar.copy(res, ps)
            nc.sync.dma_start(out=out[:, :], in_=res)
    nc.finalize()
    return nc


def _to_partition_major(flat: np.ndarray) -> np.ndarray:
    """[N, K] (f16) row-major -> [PART, N*FREE] where each partition's rows
    for consecutive N are adjacent."""
    n = flat.shape[0]
    padded = np.zeros((n, KPAD), dtype=np.float16)
    padded[:, :K] = flat
    # [n, PART, FREE] -> [PART, n, FREE] -> [PART, n*FREE]
    return np.ascontiguousarray(
        padded.reshape(n, PART, FREE).transpose(1, 0, 2)
    ).reshape(PART, n * FREE)


def make_in_maps(x: np.ndarray, W: np.ndarray, fc1_w: np.ndarray):
    x16 = np.asarray(x).astype(np.float16)
    W16 = np.asarray(W).astype(np.float16)
    f16 = np.asarray(fc1_w).astype(np.float16).reshape(T, P)
    in_maps = []
    for c in range(NCORES):
        t0 = c * TS
        xs = _to_partition_major(x16[:, t0 : t0 + TS, :].reshape(B, K))
        ws = _to_partition_major(W16[t0 : t0 + TS, :].reshape(1, K))
        fs = _to_partition_major(f16[t0 : t0 + TS, :].reshape(1, K))
        m = {"wf": np.concatenate([ws, fs], axis=1)}
        b0 = 0
        for g, nb in enumerate(CHUNKS):
            m[f"xs{g}"] = np.ascontiguousarray(
                xs[:, b0 * FREE : (b0 + nb) * FREE]
            )
            b0 += nb
        in_maps.append(m)
    return in_maps


def kernel(x, W, fc1_w, fc1_b):
    global LAST_RESULT
    nc = build_program()
    in_maps = make_in_maps(x, W, fc1_w)
    res = run_bass_kernel_spmd(
        nc, in_maps, core_ids=list(range(NCORES)), trace=TRACE
    )
    LAST_RESULT = res
    partial = np.zeros(B, dtype=np.float64)
    for r in res.results:
        partial += r["out"][0].astype(np.float64)
    out = partial.astype(np.float32) + np.float32(np.asarray(fc1_b).reshape(-1)[0])
    return out.reshape(B, 1).astype(np.float32)


# revision 6
# speedup vs baseline: 1.9204x; 1.1845x over previous
"""Trainium2 Bass kernel for nn_Net_91268055040039 (dense_mlp).

Computes out[b] = sum_{t,p} x[b,t,p] * |W[t,p]| * fc1_w[0, t*P+p] + fc1_b
  x: [32, 400, 10000] f32, W: [400, 10000] f32, fc1_w: [1, 4000000] f32.

Strategy (v5, fp16 + 3 compute lanes + tuned stream): shard T=400 into 8
slices of 50 rows. x/W/fc1 cast to fp16 on host (halves HBM traffic, the
binding resource; quantization rel-err ~4e-3 vs the 2e-2 gate). FREE padded
3907->3908 for DVE 2x-mode 4B alignment.

Stream plan (SWDGE ring, dense 2MB-max dram blocks, measured 389 GB/s):
  W(1MB) -> fc1(1MB) -> b0(1MB) -> 13x 2-batch chunks -> b27,b28,b29 single.
  ACT abs(W) overlaps the fc1 transfer; v=|W|*fc1 is ready ~when b0 lands.
  b30/b31 are prefetched CONCURRENTLY on the idle sync (HWDGE) ring and
  computed mid-stream, so the post-stream tail is one fused stt + out chain.

Compute lanes (measured: DVE TT pair 4.15us, stt 4.3us, ACT full reduce
3.83us, ACT 512-wide psum pick 1.13us, PE 8x512 matmul chain ~2-4us):
  - every batch: DVE tensor_tensor multiply (2x mode), mostly 2 batches per
    op against a replicated v to amortize op overhead
  - 18 batches: PE ones-matmul partition-reduce into a psum bank row
    (6 banks x 3 quad offsets {0,32,64}), then cheap ACT pick -> acc[q, b]
  - 11 batches: ACT full free-dim reduce -> acc[:, b]
  - b29 (last swdge batch) + b30/b31 (prefetched): fused DVE stt
acc is zeroed up front; every path leaves out[b] = colsum(acc[:, b]), so one
PE ones-matmul -> psum[1,32] -> copy -> DMA finishes the kernel.
Host sums the 8 per-core partials in f64 and adds fc1_b.
"""

import numpy as np

import concourse.bass as bass
import concourse.bacc as bacc
import concourse.mybir as mybir
from concourse.tile import TileContext
from concourse.bass_utils import run_bass_kernel_spmd

B, T, P = 32, 400, 10000
NCORES = 8
TS = T // NCORES          # 50 T-rows per core
K = TS * P                # 500000 reduction elements per core per batch
PART = 128
FREE = 3908               # ceil(K/128)=3907, padded to even for 4B alignment
KPAD = PART * FREE        # 500224 (224 zero pad)
# SWDGE x chunks cover b0..b29; b30/b31 ride the sync ring.
CHUNKS = [1] + [2] * 13 + [1, 1, 1]
ACT_BATCHES = frozenset(range(1, 22, 2))               # 11 full ACT reduces
PE_BATCHES = tuple(range(0, 29, 2)) + (23, 25, 27)     # 18 PE-reduce batches
BANK = 512                # psum bank width in f32
NPB = 6                   # psum bank tiles for PE reduces (x3 quads = 18)
F16 = mybir.dt.float16
F32 = mybir.dt.float32

# Set by the test harness to capture an NTFF profile; harmless when False.
TRACE = False
LAST_RESULT = None


def build_program() -> bass.Bass:
    # Bacc (not raw Bass): its compile() splits multi-sem waits into separate
    # instructions - this neuronxcc build allows only 1 sync-wait per inst.
    nc = bacc.Bacc()
    xcs = [
        nc.declare_dram_parameter(f"xs{g}", [PART, nb * FREE], F16, isOutput=False)
        for g, nb in enumerate(CHUNKS)
    ]
    xt30d = nc.declare_dram_parameter("x30", [PART, FREE], F16, isOutput=False)
    xt31d = nc.declare_dram_parameter("x31", [PART, FREE], F16, isOutput=False)
    wWd = nc.declare_dram_parameter("wW", [PART, FREE], F16, isOutput=False)
    wfd = nc.declare_dram_parameter("wf1", [PART, FREE], F16, isOutput=False)
    out = nc.declare_dram_parameter("out", [1, B], F32, isOutput=True)

    # 8 accumulating matmul windows covering FREE=3908 into one 512-col bank.
    # Window 0 (start=True, full width) resets every psum col; the 324-wide
    # remainder accumulates onto cols 0:324.
    wins = [(w, BANK, w == 0, False) for w in range(0, 7 * BANK, BANK)]
    wins.append((7 * BANK, FREE - 7 * BANK, False, True))

    pe_slot = {b: i for i, b in enumerate(PE_BATCHES)}

    with TileContext(nc) as tc:
        with (
            tc.tile_pool(name="const", bufs=1) as cpool,
            tc.tile_pool(name="xp", bufs=4) as xpool,
            tc.tile_pool(name="sp", bufs=3) as spool,
            tc.tile_pool(name="psum", bufs=1, space="PSUM") as ppool,
        ):
            # Params first on the SWDGE ring; abs(W) overlaps the fc1 DMA.
            # b30/b31 prefetch concurrently on the sync (HWDGE) ring.
            wWt = cpool.tile([PART, FREE], F16)
            nc.gpsimd.dma_start(out=wWt, in_=wWd[:, :])
            wft = cpool.tile([PART, FREE], F16)
            nc.gpsimd.dma_start(out=wft, in_=wfd[:, :])
            xt30 = cpool.tile([PART, FREE], F16)
            nc.sync.dma_start(out=xt30, in_=xt30d[:, :])
            xt31 = cpool.tile([PART, FREE], F16)
            nc.sync.dma_start(out=xt31, in_=xt31d[:, :])

            nc.scalar.activation(
                out=wWt, in_=wWt, func=mybir.ActivationFunctionType.Abs
            )
            # v2 = [v, v] so a paired TT can process 2 batches in one op.
            v2 = cpool.tile([PART, 2 * FREE], F16)
            v = v2[:, :FREE]
            nc.vector.tensor_tensor(
                out=v, in0=wWt, in1=wft, op=mybir.AluOpType.mult
            )
            nc.scalar.copy(v2[:, FREE:], v)

            ones = cpool.tile([PART, 1], F32)
            nc.vector.memset(ones, 1.0)
            ones16 = cpool.tile([PART, 1], F16)
            nc.vector.memset(ones16, 1.0)
            acc = cpool.tile([PART, B], F32)
            nc.vector.memset(acc, 0.0)
            dump = cpool.tile([PART, FREE], F16)
            pbank = [
                ppool.tile([PART, BANK], F32, name=f"pbank{i}")
                for i in range(NPB)
            ]

            def reduce_batch(b, sc, off):
                """Free-dim reduce of sc[:, off:off+FREE] into acc[:, b]."""
                if b in pe_slot:
                    s = pe_slot[b]
                    pt = pbank[s % NPB]
                    q = 32 * (s // NPB)
                    for w0, nw, st, sp in wins:
                        nc.tensor.matmul(
                            out=pt[q : q + 1, :nw],
                            lhsT=ones16,
                            rhs=sc[:, off + w0 : off + w0 + nw],
                            start=st,
                            stop=sp,
                            skip_group_check=True,
                        )
                    nc.scalar.activation(
                        out=dump[q : q + 1, :BANK],
                        in_=pt[q : q + 1, :],
                        func=mybir.ActivationFunctionType.Copy,
                        accum_out=acc[q : q + 1, b : b + 1],
                    )
                else:
                    nc.scalar.activation(
                        out=dump,
                        in_=sc[:, off : off + FREE],
                        func=mybir.ActivationFunctionType.Copy,
                        accum_out=acc[:, b : b + 1],
                    )

            def stt_batch(b, xin):
                sc = spool.tile([PART, 2 * FREE], F16, tag="sc")
                nc.vector.scalar_tensor_tensor(
                    out=sc[:, :FREE],
                    in0=xin,
                    scalar=0.0,
                    in1=v,
                    op0=mybir.AluOpType.bypass,
                    op1=mybir.AluOpType.mult,
                    accum_out=acc[:, b : b + 1],
                )

            b0 = 0
            for g, nb in enumerate(CHUNKS):
                xt = xpool.tile([PART, 2 * FREE], F16, tag="xt")
                nc.gpsimd.dma_start(out=xt[:, : nb * FREE], in_=xcs[g][:, :])
                if b0 + nb - 1 == 29:
                    # Last swdge batch: fused stt keeps the tail short.
                    stt_batch(29, xt[:, :FREE])
                elif nb == 2:
                    # One paired TT computes both batches' products.
                    sc = spool.tile([PART, 2 * FREE], F16, tag="sc")
                    nc.vector.tensor_tensor(
                        out=sc, in0=xt, in1=v2, op=mybir.AluOpType.mult
                    )
                    reduce_batch(b0, sc, 0)
                    reduce_batch(b0 + 1, sc, FREE)
                else:
                    sc = spool.tile([PART, 2 * FREE], F16, tag="sc")
                    nc.vector.tensor_tensor(
                        out=sc[:, :FREE], in0=xt[:, :FREE], in1=v,
                        op=mybir.AluOpType.mult,
                    )
                    reduce_batch(b0, sc, 0)
                b0 += nb
                if g == 5:
                    stt_batch(30, xt30)
                elif g == 9:
                    stt_batch(31, xt31)

            ps = ppool.tile([1, B], F32)
            nc.tensor.matmul(out=ps, lhsT=ones, rhs=acc, start=True, stop=True)
            res = cpool.tile([1, B], F32)
            nc.scalar.copy(res, ps)
            nc.sync.dma_start(out=out[:, :], in_=res)
    nc.finalize()
    return nc


def _to_partition_major(flat: np.ndarray) -> np.ndarray:
    """[N, K] (f16) row-major -> [PART, N*FREE] where each partition's rows
    for consecutive N are adjacent."""
    n = flat.shape[0]
    padded = np.zeros((n, KPAD), dtype=np.float16)
    padded[:, :K] = flat
    # [n, PART, FREE] -> [PART, n, FREE] -> [PART, n*FREE]
    return np.ascontiguousarray(
        padded.reshape(n, PART, FREE).transpose(1, 0, 2)
    ).reshape(PART, n * FREE)


def make_in_maps(x: np.ndarray, W: np.ndarray, fc1_w: np.ndarray):
    x16 = np.asarray(x).astype(np.float16)
    W16 = np.asarray(W).astype(np.float16)
    f16 = np.asarray(fc1_w).astype(np.float16).reshape(T, P)
    in_maps = []
    for c in range(NCORES):
        t0 = c * TS
        xs = _to_partition_major(x16[:, t0 : t0 + TS, :].reshape(B, K))
        m = {
            "wW": _to_partition_major(W16[t0 : t0 + TS, :].reshape(1, K)),
            "wf1": _to_partition_major(f16[t0 : t0 + TS, :].reshape(1, K)),
            "x30": np.ascontiguousarray(xs[:, 30 * FREE : 31 * FREE]),
            "x31": np.ascontiguousarray(xs[:, 31 * FREE : 32 * FREE]),
        }
        b0 = 0
        for g, nb in enumerate(CHUNKS):
            m[f"xs{g}"] = np.ascontiguousarray(
                xs[:, b0 * FREE : (b0 + nb) * FREE]
            )
            b0 += nb
        in_maps.append(m)
    return in_maps


def kernel(x, W, fc1_w, fc1_b):
    global LAST_RESULT
    nc = build_program()
    in_maps = make_in_maps(x, W, fc1_w)
    res = run_bass_kernel_spmd(
        nc, in_maps, core_ids=list(range(NCORES)), trace=TRACE
    )
    LAST_RESULT = res
    partial = np.zeros(B, dtype=np.float64)
    for r in res.results:
        partial += r["out"][0].astype(np.float64)
    out = partial.astype(np.float32) + np.float32(np.asarray(fc1_b).reshape(-1)[0])
    return out.reshape(B, 1).astype(np.float32)
